# revision 27
# baseline (speedup 1.0000x reference)
"""ChebNetwork (K=2, 4 layers) Trainium2 Bass kernel, 8-core SPMD.

Sharding: nodes partitioned across 8 cores by target range (12544 padded
rows per core).  Normalized edge weights (-dinv[src]*w*dinv[tgt]) are
precomputed on host and folded into the per-edge scatter weights.

Per layer:
  A: per 128-node group g: y = h @ W1 (PE, PSUM), cast bf16 (ACT) into a
     staging ring, stored to d_agin (bf16, quad-packed: 4 nodes per
     512-byte row).
  B: AllGather d_agin -> d_yall (bf16, all cores' y).
  C: group-major scatter: per target group g one PSUM accumulation chain
       bias-mm (1x128 ones x bias row, start) -> a-mm (h@W0) ->
       one matmul per 128-edge chunk: lhsT = M[e,n] = w'_e*[tgt_e==n]
       (built on DVE from iota/is_eq), rhs = 64-col parity slice of the
       dma_gather'ed quad rows of y[src] -> sigmoid(PSUM) on ACT.
     Edge chunks are quad-parity pure (sorted by src&3 within group) so
     the rhs slice offset is uniform per chunk.
  Last 3 layers read h via PE transpose into s_hT (bf16, feature-major).
"""
import os
import sys

sys.path.insert(0, "/opt/trn_rl_repo")

import numpy as np
import ml_dtypes

import concourse.bass as bass
import concourse.bacc as bacc
import concourse.mybir as mybir
from concourse import library_config
from concourse.bass_utils import run_bass_kernel_spmd

F32 = mybir.dt.float32
BF16 = mybir.dt.bfloat16
I16 = mybir.dt.int16

N_NODES = 100000
N_EDGES = 3200000
NCORES = 8
NLOC = 12500          # nodes owned per core
NGRP = 98             # 128-node groups per core (12544 padded)
NPAD = NGRP * 128     # 12544
NQROW = NPAD // 4     # quad rows per core slab (3136)
F1 = 128              # input feature dim
FH = 64               # hidden dim
NQ = 4                # src parity classes (quad packing)
GCH = 64              # chunks (of 128 edges) per dma_gather instruction
N_LAYERS = 4          # bisection knob
SKIP_AG = False       # bisection knob: skip collectives
SKIP_GATHER = False   # bisection knob: skip dma_gather instructions
MRING = 48            # M-matrix ring slots
NMETA = 6             # meta (idx/tgt/wts) ring slots


# ----------------------------------------------------------------------------
# host-side structure building
# ----------------------------------------------------------------------------

def _pack_chunks(vals, dtype):
    """[CH*128] -> [128, CH]: edge c*128+p at [p, c]."""
    ch = vals.shape[0] // 128
    return np.ascontiguousarray(vals.reshape(ch, 128).T.astype(dtype))


def _pack_idx(vals):
    """[CH*128] int16 -> wrapped [128, CH*8] (16-row wrap, tiled x8)."""
    n = vals.shape[0]
    w = vals.reshape(n // 16, 16).T  # [16, n/16]
    return np.ascontiguousarray(np.tile(w, (8, 1)).astype(np.int16))


def _segment_place(key, nseg, seg_counts_pad, idx, et, w):
    """Scatter edges (grouped by key) into padded per-segment slots."""
    order = np.argsort(key, kind="stable")
    key_s = key[order]
    idx_s, et_s, w_s = idx[order], et[order], w[order]
    counts = np.bincount(key_s, minlength=nseg)
    seg_start = np.zeros(nseg + 1, np.int64)
    np.cumsum(counts, out=seg_start[1:])
    pad_off = np.zeros(nseg + 1, np.int64)
    np.cumsum(seg_counts_pad * 128, out=pad_off[1:])
    total = int(pad_off[-1])
    rank = np.arange(key_s.shape[0], dtype=np.int64) - seg_start[key_s]
    dest = pad_off[key_s] + rank
    idx_arr = np.zeros(total, np.int16)
    tgt_arr = np.zeros(total, np.float32)
    w_arr = np.zeros(total, np.float32)
    idx_arr[dest] = idx_s
    tgt_arr[dest] = et_s
    w_arr[dest] = w_s
    return idx_arr, tgt_arr, w_arr


def build_structure(x, edge_index, edge_weight, Ws, bs):
    src = np.asarray(edge_index[0]).astype(np.int64)
    tgt = np.asarray(edge_index[1]).astype(np.int64)
    ew = np.asarray(edge_weight).astype(np.float32)
    x = np.asarray(x).astype(np.float32)

    # symmetric normalization on host (deg over source index, as reference)
    deg = np.zeros(N_NODES, np.float32)
    np.add.at(deg, src, ew)
    dinv = np.where(deg > 0,
                    1.0 / np.sqrt(np.maximum(deg, 1e-12)), 0.0
                    ).astype(np.float32)
    wp = (-dinv[src] * ew * dinv[tgt]).astype(np.float32)

    NSEG = NGRP * NQ
    cnt = np.zeros((NCORES, NSEG), np.int64)
    per_core = []
    for c in range(NCORES):
        c0 = c * NLOC
        sel = (tgt >= c0) & (tgt < c0 + NLOC)
        es, et, w = src[sel], tgt[sel] - c0, wp[sel]
        b = es // NLOC
        sr = es - b * NLOC
        qidx = b * NQROW + (sr >> 2)       # global quad row in d_yall
        qq = sr & 3                        # parity class
        g = et >> 7
        key = g * NQ + qq
        per_core.append((qidx, (et & 127).astype(np.float32), w, key))
        cnt[c] = np.bincount(key, minlength=NSEG)

    seg_ch = -(-cnt.max(axis=0) // 128)    # [NSEG] chunks per (g, q)
    ng = seg_ch.reshape(NGRP, NQ).sum(axis=1)  # chunks per group
    assert ng.min() >= 1, "empty target group: no PSUM chain would close"
    CHT = int(ng.sum())

    # chunk schedule: (group, parity, first_of_group, last_of_group)
    chunks = []
    for g in range(NGRP):
        n = int(ng[g])
        k = 0
        for q in range(NQ):
            for _ in range(int(seg_ch[g * NQ + q])):
                chunks.append((g, q, k == 0, k == n - 1))
                k += 1
    assert len(chunks) == CHT

    # constants / weights
    iota = np.ascontiguousarray(np.broadcast_to(
        np.arange(128, dtype=ml_dtypes.bfloat16), (128, 128)))
    ident = np.eye(128, dtype=np.float32)
    # partition-0 ones row: bias matmul lhsT (out[n,f] += sum_p [p==0]*bt[p,f])
    ones1 = np.zeros((128, 128), np.float32)
    ones1[0, :] = 1.0
    wf = np.zeros((128, 128), np.float32)
    wf[:, 0:64] = Ws[0][1]
    wf[:, 64:128] = Ws[0][0]
    wb = np.zeros((64, 384), ml_dtypes.bfloat16)
    for l in (1, 2, 3):
        wb[:, (l - 1) * 128:(l - 1) * 128 + 64] = Ws[l][1].astype(
            ml_dtypes.bfloat16)
        wb[:, (l - 1) * 128 + 64:(l - 1) * 128 + 128] = Ws[l][0].astype(
            ml_dtypes.bfloat16)
    btile = np.zeros((128, 256), np.float32)
    for l in range(4):
        btile[0, l * 64:(l + 1) * 64] = bs[l]

    in_maps = []
    for c in range(NCORES):
        qidx, et, w, key = per_core[c]
        idx_a, tgt_a, w_a = _segment_place(key, NSEG, seg_ch, qidx, et, w)
        xT = np.zeros((128, NPAD), np.float32)
        xT[:, :NLOC] = x[c * NLOC:(c + 1) * NLOC].T
        in_maps.append({
            "xT": xT,
            "idx": _pack_idx(idx_a),
            "tgt": _pack_chunks(tgt_a, np.float32),
            "wts": _pack_chunks(w_a, np.float32),
            "iota": np.asarray(iota),
            "ident": np.asarray(ident),
            "ones1": ones1,
            "wf": wf,
            "wb": np.asarray(wb),
            "btile": btile,
        })

    S = dict(CHT=CHT, chunks=chunks)
    return S, in_maps


# ----------------------------------------------------------------------------
# program generation
# ----------------------------------------------------------------------------

class Emitter:
    """Records per-engine op closures with exact semaphore thresholds."""

    def __init__(self):
        self.ops = {k: [] for k in ("SP", "POOL", "PE", "DVE", "ACT")}
        self.pe = 0
        self.dve = 0
        self.act = 0
        self.ld = 0    # count of SP DMAs
        self.st = 0    # count of ACT-issued store DMAs
        self.gth = 0   # count of pool gather DMAs
        self.ag = 0

    def op(self, eng, fn):
        self.ops[eng].append(fn)


def build_program(S):
    AT = mybir.ActivationFunctionType
    is_eq = mybir.AluOpType.is_equal
    mult = mybir.AluOpType.mult

    CHT = S["CHT"]
    chunks = S["chunks"]
    NSUB = -(-CHT // GCH)

    nc = bacc.Bacc("TRN2")

    # ---- DRAM tensors
    d_xT = nc.dram_tensor("xT", [128, NPAD], F32, kind="ExternalInput")
    d_idx = nc.dram_tensor("idx", [128, CHT * 8], I16, kind="ExternalInput")
    d_tgt = nc.dram_tensor("tgt", [128, CHT], F32, kind="ExternalInput")
    d_wts = nc.dram_tensor("wts", [128, CHT], F32, kind="ExternalInput")
    d_iota = nc.dram_tensor("iota", [128, 128], BF16, kind="ExternalInput")
    d_ident = nc.dram_tensor("ident", [128, 128], F32, kind="ExternalInput")
    d_ones1 = nc.dram_tensor("ones1", [128, 128], F32, kind="ExternalInput")
    d_wf = nc.dram_tensor("wf", [128, 128], F32, kind="ExternalInput")
    d_wb = nc.dram_tensor("wb", [64, 384], BF16, kind="ExternalInput")
    d_bt = nc.dram_tensor("btile", [128, 256], F32, kind="ExternalInput")
    d_out = nc.dram_tensor("out", [NPAD, FH], F32, kind="ExternalOutput")
    d_agin = nc.dram_tensor("agin", [NPAD, FH], BF16)
    d_yall = nc.dram_tensor("yall", [NCORES * NPAD, FH], BF16,
                            addr_space="Shared")

    E = Emitter()

    from contextlib import ExitStack
    with ExitStack() as _st:
        s_xT = _st.enter_context(nc.sbuf_tensor("s_xT", [128, NPAD], F32))
        s_hT = _st.enter_context(nc.sbuf_tensor("s_hT", [64, NPAD], BF16))
        s_iota = _st.enter_context(nc.sbuf_tensor("s_iota", [128, 128], BF16))
        s_ident = _st.enter_context(
            nc.sbuf_tensor("s_ident", [128, 128], F32))
        s_ones1 = _st.enter_context(
            nc.sbuf_tensor("s_ones1", [128, 128], F32))
        s_wf = _st.enter_context(nc.sbuf_tensor("s_wf", [128, 128], F32))
        s_wb = _st.enter_context(nc.sbuf_tensor("s_wb", [64, 384], BF16))
        s_bt = _st.enter_context(nc.sbuf_tensor("s_bt", [128, 256], F32))
        s_idx = _st.enter_context(
            nc.sbuf_tensor("s_idx", [128, NMETA * GCH * 8], I16))
        s_tgt = _st.enter_context(
            nc.sbuf_tensor("s_tgt", [128, NMETA * GCH], F32))
        s_wts = _st.enter_context(
            nc.sbuf_tensor("s_wts", [128, NMETA * GCH], F32))
        s_gath = _st.enter_context(
            nc.sbuf_tensor("s_gath", [128, 2 * GCH, 4 * FH], BF16))
        s_m = _st.enter_context(nc.sbuf_tensor("s_m", [128, MRING, 128], BF16))
        s_yst = _st.enter_context(nc.sbuf_tensor("s_yst", [128, 16, FH], BF16))
        s_h = _st.enter_context(nc.sbuf_tensor("s_h", [128, 4, FH], F32))
        s_ho = _st.enter_context(nc.sbuf_tensor("s_ho", [128, 2, 8, FH], F32))
        p_y = [_st.enter_context(nc.psum_tensor(f"p_y{i}", [128, 512], F32))
               for i in range(2)]
        p_acc = [_st.enter_context(nc.psum_tensor(f"p_a{i}", [128, 512], F32))
                 for i in range(4)]
        p_t = [_st.enter_context(nc.psum_tensor(f"p_t{i}", [128, 512], F32))
               for i in range(2)]
        q_pe = _st.enter_context(nc.semaphore("q_pe"))
        q_dve = _st.enter_context(nc.semaphore("q_dve"))
        q_act = _st.enter_context(nc.semaphore("q_act"))
        q_ld = [_st.enter_context(nc.semaphore(f"q_ld{k}")) for k in range(4)]
        q_gth = [_st.enter_context(nc.semaphore(f"q_gth{k}"))
                 for k in range(2)]
        q_st = _st.enter_context(nc.semaphore("q_st"))
        q_ag = _st.enter_context(nc.semaphore("q_ag"))
        block = _st.enter_context(nc.Block())

        # quad-row view of the gathered table
        yall_q = d_yall[:].rearrange("(r k) f -> r (k f)", k=4)

        # ------- tracked ring state (values recorded at emission time)
        mring_free = [0] * MRING      # q_pe value freeing the M slot
        gslot_free = [0, 0]           # q_pe value freeing gather dst slot
        meta_free_dve = [0] * NMETA   # q_dve value freeing meta slot
        meta_free_gth = [0] * NMETA   # q_gth idx freeing meta slot (idx read)
        yfree = [0, 0]                # q_act value freeing p_y slot
        accfree = [0, 0, 0, 0]        # q_act value freeing p_acc slot
        ptfree = [0, 0]               # q_dve value freeing p_t slot
        shfree = [0, 0, 0, 0]         # q_pe value freeing s_h slot
        yst_free = [0] * 16           # q_ld idx freeing s_yst slot
        sho_free = [0, 0]             # q_ld idx freeing s_ho slot
        hT_val = [0] * NGRP           # q_dve value of hT copy per group

        def sp(fn):
            E.op("SP", fn)

        def pe(fn):
            E.op("PE", fn)

        def dve(fn):
            E.op("DVE", fn)

        def act(fn):
            E.op("ACT", fn)

        def pool(fn):
            E.op("POOL", fn)

        # --- rotating DMA sem helpers (1 in-flight per sem, race-free) ---
        def sp_dma(idx, out_ap, in_ap):
            """Issue SP DMA with 1-based global index idx."""
            k = (idx - 1) % 4
            if idx > 4:
                nc.sync.wait_ge(q_ld[k], 16 * ((idx - 1) // 4))
            nc.sync.dma_start(out_ap, in_ap).then_inc(q_ld[k], 16)

        def ld_wait_one(ns, idx):
            if idx <= 0:
                return
            k = (idx - 1) % 4
            ns.wait_ge(q_ld[k], 16 * ((idx - 1) // 4 + 1))

        def ld_wait_all(ns, idx):
            for k in range(4):
                c = (idx - 1 - k) // 4 + 1 if idx - 1 >= k else 0
                if c:
                    ns.wait_ge(q_ld[k], 16 * c)

        def gth_wait_one(ns, j):
            if j <= 0:
                return
            k = (j - 1) % 2
            ns.wait_ge(q_gth[k], 16 * ((j - 1) // 2 + 1))

        def gth_wait_all(ns, j):
            for k in range(2):
                c = (j - 1 - k) // 2 + 1 if j - 1 >= k else 0
                if c:
                    ns.wait_ge(q_gth[k], 16 * c)

        def st_wait(ns, j):
            if j > 0:
                ns.wait_ge(q_st, 16 * j)

        # =================== prologue: constants ===========================
        def f_consts():
            for j, (dst, src_) in enumerate((
                    (s_iota, d_iota), (s_ident, d_ident), (s_ones1, d_ones1),
                    (s_wf, d_wf), (s_wb, d_wb), (s_bt, d_bt), (s_xT, d_xT))):
                sp_dma(j + 1, dst[:], src_[:])
        sp(f_consts)
        E.ld += 7
        XT_LD = 7   # sp_dma index of the xT load

        # =================== layers ========================================
        for l in range(N_LAYERS):
            if l == 0:
                def lhsT_of(g):
                    return s_xT[:, g * 128:(g + 1) * 128]
                rhs1 = s_wf[:, 0:64]
                rhs0 = s_wf[:, 64:128]
            else:
                def lhsT_of(g):
                    return s_hT[:, g * 128:(g + 1) * 128]
                rhs1 = s_wb[:, (l - 1) * 128:(l - 1) * 128 + 64]
                rhs0 = s_wb[:, (l - 1) * 128 + 64:(l - 1) * 128 + 128]

            # ---------------- phase A: y = h @ W1, store bf16 ------------
            act_yst = [0] * NGRP
            for g in range(NGRP):
                yq = g % 2
                yneed = yfree[yq]
                lhs_dve = 0 if l == 0 else hT_val[g]

                def f_y(g=g, yq=yq, yneed=yneed, lhs_dve=lhs_dve, l=l,
                        lhsT_ap=lhsT_of(g), rhs1=rhs1):
                    if l == 0:
                        ld_wait_one(nc.tensor, XT_LD)
                        ld_wait_one(nc.tensor, 4)
                    else:
                        ld_wait_one(nc.tensor, 5)
                        nc.tensor.wait_ge(q_dve, lhs_dve)
                    if yneed:
                        nc.tensor.wait_ge(q_act, yneed)
                    nc.tensor.matmul(
                        p_y[yq][:, 0:64], lhsT_ap, rhs1,
                        start=True, stop=True).then_inc(q_pe, 1)
                pe(f_y)
                E.pe += 1
                y_pe = E.pe

                ys = g % 16
                ystneed = yst_free[ys]
                do_store = (g % 8 == 7 or g == NGRP - 1)
                g0 = g - (g % 8)
                nb = g - g0 + 1

                def f_yc(g=g, yq=yq, ys=ys, y_pe=y_pe, ystneed=ystneed,
                         do_store=do_store, g0=g0, nb=nb, l=l,
                         agw=(E.ag if l > 0 else 0)):
                    nc.scalar.wait_ge(q_pe, y_pe)
                    if ystneed:
                        st_wait(nc.scalar, ystneed)
                    nc.scalar.activation(
                        s_yst[:, ys, :], p_y[yq][:, 0:64],
                        AT.Copy).then_inc(q_act, 1)
                    if do_store:
                        if agw:
                            nc.scalar.wait_ge(q_ag, agw)
                        nc.scalar.dma_start(
                            d_agin[g0 * 128:(g0 + nb) * 128, :].rearrange(
                                "(a p) f -> p a f", p=128),
                            s_yst[:, g0 % 16:g0 % 16 + nb, :]
                        ).then_inc(q_st, 16)
                act(f_yc)
                E.act += 1
                yfree[yq] = E.act
                act_yst[g] = E.act
                if do_store:
                    E.st += 1
                    for gg in range(g0, g0 + nb):
                        yst_free[gg % 16] = E.st

            # ---------------- phase B: allgather ----------------
            yst_all = E.st
            gth_before = E.gth

            def f_ag(yst_all=yst_all, gth_before=gth_before):
                st_wait(nc.gpsimd, yst_all)
                if gth_before:
                    gth_wait_all(nc.gpsimd, gth_before)
                nc.gpsimd.collective_compute(
                    "AllGather", mybir.AluOpType.bypass,
                    replica_groups=[list(range(NCORES))],
                    ins=[d_agin[:]], outs=[d_yall[:]],
                ).then_inc(q_ag, 1)
            if not SKIP_AG:
                pool(f_ag)
                E.ag += 1
            ag_now = E.ag

            # ---------------- phase C: group-major scatter ----------------
            gi = 0           # global chunk index within layer
            cur_sub = -1
            sub_gw = 0       # gather idx of current sub
            for ci, (g, qq, first, last) in enumerate(chunks):
                si = ci // GCH
                col = ci % GCH
                if si != cur_sub:
                    cur_sub = si
                    nch = min(GCH, CHT - si * GCH)
                    ms = si % NMETA
                    # meta loads for this sub
                    ndve = meta_free_dve[ms]
                    ngth = meta_free_gth[ms]

                    def f_mld(si=si, nch=nch, ms=ms, ndve=ndve, ngth=ngth,
                              i0=E.ld):
                        if ndve:
                            nc.sync.wait_ge(q_dve, ndve)
                        if ngth:
                            gth_wait_all(nc.sync, ngth)
                        c0 = si * GCH
                        sp_dma(i0 + 1,
                               s_idx[:, ms * GCH * 8:ms * GCH * 8 + nch * 8],
                               d_idx[:, c0 * 8:(c0 + nch) * 8])
                        sp_dma(i0 + 2,
                               s_tgt[:, ms * GCH:ms * GCH + nch],
                               d_tgt[:, c0:c0 + nch])
                        sp_dma(i0 + 3,
                               s_wts[:, ms * GCH:ms * GCH + nch],
                               d_wts[:, c0:c0 + nch])
                    sp(f_mld)
                    E.ld += 3
                    meta_ld = E.ld

                    gs = si % 2
                    gneed = gslot_free[gs]

                    def f_g(si=si, nch=nch, ms=ms, gs=gs, gneed=gneed,
                            meta_ld=meta_ld, first_sub=(si == 0),
                            ag_now=ag_now, E0=E.gth):
                        if first_sub and ag_now:
                            nc.gpsimd.wait_ge(q_ag, ag_now)
                        ld_wait_all(nc.gpsimd, meta_ld)
                        if gneed:
                            nc.gpsimd.wait_ge(q_pe, gneed)
                        nc.gpsimd.dma_gather(
                            s_gath[:, gs * GCH:gs * GCH + nch, :],
                            yall_q,
                            s_idx[:, ms * GCH * 8:ms * GCH * 8 + nch * 8],
                            nch * 128, nch * 128, 4 * FH,
                            single_packet=False,
                        ).then_inc(q_gth[E0 % 2], 16)
                    if not SKIP_GATHER:
                        pool(f_g)
                        E.gth += 1
                    sub_gw = E.gth
                    new_sub = True
                else:
                    ms = si % NMETA
                    gs = si % 2
                    new_sub = False

                if first:
                    # start the PSUM chain for this group: bias then dense
                    aq = g % 4
                    aneed = accfree[aq]

                    def f_bias(aq=aq, aneed=aneed, l=l):
                        ld_wait_one(nc.tensor, 3)
                        ld_wait_one(nc.tensor, 6)
                        if aneed:
                            nc.tensor.wait_ge(q_act, aneed)
                        nc.tensor.matmul(
                            p_acc[aq][:, 0:64], s_ones1[:],
                            s_bt[:, l * 64:(l + 1) * 64],
                            start=True, stop=False).then_inc(q_pe, 1)
                    pe(f_bias)
                    E.pe += 1

                    def f_a(aq=aq, lhsT_ap=lhsT_of(g), rhs0=rhs0):
                        nc.tensor.matmul(
                            p_acc[aq][:, 0:64], lhsT_ap, rhs0,
                            start=False, stop=False).then_inc(q_pe, 1)
                    pe(f_a)
                    E.pe += 1
                else:
                    aq = g % 4

                # M-build for this chunk
                slot = gi % MRING
                mneed = mring_free[slot]
                mcol = ms * GCH + col
                meta_now = E.ld

                def f_m(slot=slot, mcol=mcol, mneed=mneed,
                        meta_now=meta_now):
                    ld_wait_all(nc.vector, meta_now)
                    if mneed:
                        nc.vector.wait_ge(q_pe, mneed)
                    nc.vector.tensor_scalar(
                        s_m[:, slot, :], s_iota[:],
                        s_tgt[:, mcol:mcol + 1], s_wts[:, mcol:mcol + 1],
                        is_eq, mult).then_inc(q_dve, 1)
                dve(f_m)
                E.dve += 1
                mwait = E.dve

                def f_mm(slot=slot, aq=aq, gs=gs, col=col, qq=qq, last=last,
                         mwait=mwait, gw=(sub_gw if new_sub else 0)):
                    nc.tensor.wait_ge(q_dve, mwait)
                    if gw:
                        gth_wait_one(nc.tensor, gw)
                    nc.tensor.matmul(
                        p_acc[aq][:, 0:64], s_m[:, slot, :],
                        s_gath[:, gs * GCH + col, qq * 64:(qq + 1) * 64],
                        start=False, stop=last).then_inc(q_pe, 1)
                pe(f_mm)
                E.pe += 1
                mring_free[slot] = E.pe
                gslot_free[gs] = E.pe
                gi += 1
                meta_free_dve[ms] = E.dve
                meta_free_gth[ms] = E.gth

                if last:
                    stop_pe = E.pe
                    if l < N_LAYERS - 1:
                        hs = g % 4
                        shneed = shfree[hs]

                        def f_sig(g=g, aq=aq, hs=hs, stop_pe=stop_pe,
                                  shneed=shneed):
                            nc.scalar.wait_ge(q_pe, stop_pe)
                            if shneed:
                                nc.scalar.wait_ge(q_pe, shneed)
                            nc.scalar.activation(
                                s_h[:, hs, :], p_acc[aq][:, 0:64],
                                AT.Sigmoid).then_inc(q_act, 1)
                        act(f_sig)
                        E.act += 1
                        accfree[aq] = E.act
                        sig_act = E.act

                        tq = g % 2
                        tneed = ptfree[tq]

                        def f_tr(hs=hs, tq=tq, tneed=tneed, sig_act=sig_act):
                            ld_wait_one(nc.tensor, 2)
                            nc.tensor.wait_ge(q_act, sig_act)
                            if tneed:
                                nc.tensor.wait_ge(q_dve, tneed)
                            nc.tensor.transpose(
                                p_t[tq][0:64, 0:128], s_h[:, hs, :],
                                s_ident[:]).then_inc(q_pe, 1)
                        pe(f_tr)
                        E.pe += 1
                        shfree[hs] = E.pe
                        tr_pe = E.pe

                        def f_hc(g=g, tq=tq, tr_pe=tr_pe):
                            nc.vector.wait_ge(q_pe, tr_pe)
                            nc.vector.tensor_copy(
                                s_hT[:, g * 128:(g + 1) * 128],
                                p_t[tq][0:64, 0:128]).then_inc(q_dve, 1)
                        dve(f_hc)
                        E.dve += 1
                        ptfree[tq] = E.dve
                        hT_val[g] = E.dve
                    else:
                        os_ = (g // 8) % 2
                        oneed = sho_free[os_]
                        do_store = (g % 8 == 7 or g == NGRP - 1)
                        g0 = g - (g % 8)
                        nb = g - g0 + 1

                        def f_sig(g=g, aq=aq, os_=os_, stop_pe=stop_pe,
                                  oneed=oneed, first_of_batch=(g % 8 == 0),
                                  do_store=do_store, g0=g0, nb=nb):
                            nc.scalar.wait_ge(q_pe, stop_pe)
                            if first_of_batch and oneed:
                                st_wait(nc.scalar, oneed)
                            nc.scalar.activation(
                                s_ho[:, os_, g % 8, :], p_acc[aq][:, 0:64],
                                AT.Sigmoid).then_inc(q_act, 1)
                            if do_store:
                                nc.scalar.dma_start(
                                    d_out[g0 * 128:(g0 + nb) * 128, :
                                          ].rearrange("(a p) f -> p a f",
                                                      p=128),
                                    s_ho[:, os_, 0:nb, :]).then_inc(q_st, 16)
                        act(f_sig)
                        E.act += 1
                        accfree[aq] = E.act
                        if do_store:
                            E.st += 1
                            sho_free[os_] = E.st

        # final waits
        final_ld = E.ld
        final_gth = E.gth
        final_st = E.st

        def f_fin():
            ld_wait_all(nc.sync, final_ld)
        sp(f_fin)

        def f_fin_g():
            gth_wait_all(nc.gpsimd, final_gth)
        pool(f_fin_g)

        def f_fin_a():
            st_wait(nc.scalar, final_st)
        act(f_fin_a)

        # ------------- emit engine programs -------------
        @block.sync
        def _(eng):
            for fn in E.ops["SP"]:
                fn()

        @block.gpsimd
        def _(eng):
            eng.load_library(library_config.mlp)
            for fn in E.ops["POOL"]:
                fn()

        @block.vector
        def _(eng):
            for fn in E.ops["DVE"]:
                fn()

        @block.scalar
        def _(eng):
            for fn in E.ops["ACT"]:
                fn()

        @block.tensor
        def _(eng):
            for fn in E.ops["PE"]:
                fn()

    nc.compile()
    return nc


# ----------------------------------------------------------------------------
# public entry point
# ----------------------------------------------------------------------------

LAST_EXEC_NS = None
LAST_TRACE = None


def kernel(x, edge_index, edge_weight, W1, b1, W2, b2, W3, b3, W4, b4):
    Ws = [np.asarray(W, np.float32) for W in (W1, W2, W3, W4)]
    bs = [np.asarray(b, np.float32) for b in (b1, b2, b3, b4)]
    S, in_maps = build_structure(x, edge_index, edge_weight, Ws, bs)
    nc = build_program(S)
    res = run_bass_kernel_spmd(nc, in_maps, list(range(NCORES)))
    out = np.concatenate(
        [res.results[c]["out"][:NLOC] for c in range(NCORES)], axis=0)
    return np.ascontiguousarray(out.astype(np.float32))
